# revision 21
# baseline (speedup 1.0000x reference)
"""Multi-head self-attention Trainium2 kernel (8-core SPMD, full IO).

Problem: x:(2,2048,1024) f32; Wq/Wk/Wv/Wo:(1024,1024); bo:(1024,)
  out = softmax((xWq)(xWk)^T / 8) (xWv) reshaped @ Wo + bo

Sharding: data parallel on batch N=2 x tensor parallel on 16 heads in
4 groups of 4 heads.  Core c handles batch c//4, heads [4*(c%4), 4*(c%4)+4).
Each core computes a partial fc_out product (2048,1024) in bf16; the host
sums the 4 head-group partials per batch in f32 and adds the bias.

v2 schedule notes (why the emission order looks scrambled):
  The scalar engine (ACT) is the kernel's pacer: 128 exp instructions of
  [128,1024] ~= 142us that cannot run anywhere else.  Everything is arranged
  to keep ACT saturated from ~15us onward:
    - attention runs qc-outer (q-chunk 0 fully finishes early) so its
      softmax-normalize + fc_out partial product hide under q-chunk 1;
    - remaining K/V projections are interleaved between the first head-pair's
      score matmuls (PE has slack, ACT does not);
    - fc_out for qc0 is wedged into the scores psum ring mid-way through
      q-chunk 1 (scores run ahead of AV; the 16-deep es pool absorbs the lag);
    - scores are computed TRANSPOSED: S^T[k,q] so exp runs on ACT directly
      from PSUM and the softmax denominator falls out of the ones-column of V
      during the O^T accumulation (row 64 of the [65,1024] psum).
  No max subtraction: scores are ~N(0,1), bounded well inside fp32 exp range.
"""

import os

import numpy as np

import concourse.bass as bass
import concourse.tile as tile
from concourse import bacc, mybir
from concourse import bass_utils

F32 = mybir.dt.float32
BF16 = mybir.dt.bfloat16

EMBED = 1024
SEQ = 2048
NB = 2  # batch
HEADS = 16
D = 64  # head dim
NCORES = 8
GROUPS = 4  # head groups (tensor parallel)
HG = HEADS // GROUPS  # heads per core = 4
DG = HG * D  # dims per core = 256

_MM_DTYPE_NAME = "bfloat16"  # read by test.py
MD = BF16

# set by kernel(); test.py reads exec_time_ns from here
LAST_RESULTS = None
_CACHED_NC = {}

KC = EMBED // 128  # 8 contraction chunks for projections
TCH = 512  # projection token chunk
QC = 1024  # attention q-chunk (one exp instruction per head per m)
NM = SEQ // 128  # 16 key chunks in the attention m-loop
AV_LAG = 3  # AV trails scores by this many m-iterations


def build_nc():
    nc = bacc.Bacc("TRN2", target_bir_lowering=False, debug=False,
                   num_devices=NCORES)

    xT = nc.dram_tensor("xT", (EMBED, SEQ), BF16, kind="ExternalInput").ap()
    wq = nc.dram_tensor("wq", (EMBED, DG), BF16, kind="ExternalInput").ap()
    wk = nc.dram_tensor("wk", (EMBED, DG), BF16, kind="ExternalInput").ap()
    wv = nc.dram_tensor("wv", (EMBED, DG), BF16, kind="ExternalInput").ap()
    wo = nc.dram_tensor("wo", (DG, EMBED), BF16, kind="ExternalInput").ap()
    y = nc.dram_tensor("y", (SEQ, EMBED), BF16, kind="ExternalOutput").ap()
    # DRAM bounce buffers for the softmax denominators: SBUF sources can't be
    # partition-broadcast by DMA, DRAM sources can.
    den_dram = nc.dram_tensor("den_scratch", (2, HG, SEQ // 2), BF16).ap()
    rden_dram = nc.dram_tensor("rden_scratch", (2, HG, SEQ // 2), BF16).ap()

    with tile.TileContext(nc) as tc:
        with (
            tc.tile_pool(name="weights", bufs=1) as wpool,
            tc.tile_pool(name="qk", bufs=1) as qkpool,
            tc.tile_pool(name="vpool", bufs=1) as vpool,
            tc.tile_pool(name="otpool", bufs=1) as otpool,
            tc.tile_pool(name="xchunk", bufs=1) as xpool,
            tc.tile_pool(name="epool", bufs=34) as epool,
            tc.tile_pool(name="stage", bufs=2) as stpool,
            tc.tile_pool(name="den", bufs=2) as denpool,
            tc.tile_pool(name="rbc", bufs=2) as rbcpool,
            tc.tile_pool(name="ystage", bufs=4) as ypool,
            tc.tile_pool(name="psum", bufs=2, space="PSUM") as pspool,
            tc.tile_pool(name="psum_o", bufs=2, space="PSUM") as popool,
        ):
            # ---- weights (wk first: the first score matmul needs K only) ----
            wk_sb = wpool.tile([128, KC, DG], MD)
            wq_sb = wpool.tile([128, KC, DG], MD)
            wv_sb = wpool.tile([128, KC, DG], MD)
            wo_sb = wpool.tile([128, DG // 128, EMBED], MD)
            # (p c) split of the contraction index: per-partition DMA reads are
            # contiguous DRAM rows; the same permutation on both matmul
            # operands leaves the contraction sum unchanged.
            nc.gpsimd.dma_start(out=wk_sb, in_=wk.rearrange("(p c) n -> p c n", p=128))

            QTs = [qkpool.tile([128, 2, TCH], MD, name=f"qt{t}", tag=f"qt{t}")
                   for t in range(4)]
            KTs = [qkpool.tile([128, 2, TCH], MD, name=f"kt{t}", tag=f"kt{t}")
                   for t in range(4)]
            Vs = [vpool.tile([128, 4, HG, D + 1], MD, name=f"v{t}", tag=f"v{t}")
                  for t in range(4)]
            for t in range(4):
                nc.vector.memset(Vs[t][:, :, :, D:D + 1], 1.0)

            xTr = xT.rearrange("(p c) s -> p c s", p=128)
            xbigs = {}

            def load_x(tcb, split=False):
                xc = xpool.tile([128, KC, TCH], MD, name=f"xc{tcb}")
                src_ = xTr[:, :, tcb * TCH:(tcb + 1) * TCH]
                if split:
                    nc.sync.dma_start(out=xc[:, 0:KC // 2], in_=src_[:, 0:KC // 2])
                    nc.gpsimd.dma_start(out=xc[:, KC // 2:], in_=src_[:, KC // 2:])
                else:
                    nc.gpsimd.dma_start(out=xc, in_=src_)
                xbigs[tcb] = xc

            def xa(tcb):
                return xbigs[tcb]

            open_ps = {}

            def proj_qk_half(wsb, dst, tcb, mt, half):
                """Half of a Q^T/K^T projection (4 of 8 contraction chunks).
                Splitting keeps each wedged PE burst under one slot's slack."""
                key = (id(dst), tcb, mt)
                if half == 0:
                    open_ps[key] = pspool.tile([128, QC], F32, name="ps",
                                               tag="ps")
                ps = open_ps[key]
                for kc in range(half * 4, half * 4 + 4):
                    nc.tensor.matmul(
                        ps[:, 0:TCH],
                        wsb[:, kc, mt * 128:(mt + 1) * 128],
                        xa(tcb)[:, kc, :],
                        start=(kc == 0),
                        stop=(kc == KC - 1),
                    )
                if half == 1:
                    nc.vector.tensor_copy(out=dst[tcb][:, mt, :],
                                          in_=ps[:, 0:TCH])
                    del open_ps[key]

            def proj_qk(wsb, dst, tcb, mt):
                proj_qk_half(wsb, dst, tcb, mt, 0)
                proj_qk_half(wsb, dst, tcb, mt, 1)

            def proj_v(tcb, ti):
                """Project one 128-token row of V: [tokens 128, hg, d]."""
                ps = pspool.tile([128, QC], F32, name="psv", tag="ps")
                for kc in range(KC):
                    nc.tensor.matmul(
                        ps[:, 0:DG],
                        xa(tcb)[:, kc, ti * 128:(ti + 1) * 128],
                        wv_sb[:, kc, :],
                        start=(kc == 0),
                        stop=(kc == KC - 1),
                    )
                nc.vector.tensor_copy(
                    out=Vs[tcb][:, ti, :, 0:D],
                    in_=ps[:, 0:DG].rearrange("p (h d) -> p h d", h=HG))

            # ---- head: PE warmup + x0 + K0/Q0/Q1 (hm=0 halves only) so the
            # first exp fires ASAP; everything else streams in behind.
            wrm = wpool.tile([64, 64], MD)
            nc.vector.memset(wrm, 0.0)
            wps = pspool.tile([128, QC], F32, name="wps", tag="ps")
            for i in range(36):
                nc.tensor.matmul(wps[0:64, 0:64], wrm, wrm, start=True, stop=True)
            load_x(0, split=True)
            nc.sync.dma_start(out=wq_sb, in_=wq.rearrange("(p c) n -> p c n", p=128))
            load_x(1, split=True)
            load_x(2)
            load_x(3)
            nc.sync.dma_start(out=wv_sb, in_=wv.rearrange("(p c) n -> p c n", p=128))
            nc.sync.dma_start(out=wo_sb, in_=wo.rearrange("(c p) n -> p c n", p=128))
            proj_qk(wk_sb, KTs, 0, 0)
            proj_qk(wq_sb, QTs, 0, 0)
            proj_qk(wq_sb, QTs, 1, 0)

            # OT2[p, hm, q]: partition p = 64*j + d for head h = 2*hm + j.
            # This matches wo_sb's row layout so fc_out contracts K=128/pair.
            OT2 = otpool.tile([128, 2, SEQ], MD)

            def scores(hm, qc, m):
                """S^T for key chunk m, both heads of the pair, + exp -> es.
                ha-major emission: the j0/j1 matmuls land in different PE
                row-groups (partitions 0-63 vs 64-127), giving the engine a
                chance to overlap them."""
                pss = [pspool.tile([128, QC], F32, name="ps", tag="ps")
                       for _ in range(2)]
                for ha in range(QC // TCH):
                    for j in range(2):
                        nc.tensor.matmul(
                            pss[j][:, ha * TCH:(ha + 1) * TCH],
                            KTs[m // 4][j * D:(j + 1) * D, hm,
                                        (m % 4) * 128:(m % 4 + 1) * 128],
                            QTs[(QC // TCH) * qc + ha][j * D:(j + 1) * D, hm, :],
                            start=True,
                            stop=True,
                        )
                es = []
                for j in range(2):
                    e = epool.tile([128, QC], MD, name="es")
                    nc.scalar.activation(
                        out=e, in_=pss[j],
                        func=mybir.ActivationFunctionType.Exp,
                        scale=1.0 / np.sqrt(D),
                    )
                    es.append(e)
                return es

            def av(po, hm, m, es):
                for j in range(2):
                    for ha in range(QC // TCH):
                        nc.tensor.matmul(
                            po[j][:, ha * TCH:(ha + 1) * TCH],
                            Vs[m // 4][:, m % 4, 2 * hm + j, :],
                            es[j][:, ha * TCH:(ha + 1) * TCH],
                            start=(m == 0),
                            stop=(m == NM - 1),
                        )

            def finish_pair(po, hm, qc):
                """po -> OT2 rows (bf16) + denominator row -> DRAM."""
                qs = slice(qc * QC, (qc + 1) * QC)
                for j in range(2):
                    st = stpool.tile([D + 1, QC], MD, name="st", tag="st")
                    nc.vector.tensor_copy(out=st, in_=po[j][0:D + 1, :])
                    if j == 0:
                        # partition-aligned: row 64 (den) DMAs out, dims DMA'd
                        # with the j=1 hop below would also work but DVE->OT2
                        # direct is cheaper for j=0.
                        nc.gpsimd.dma_start(out=OT2[0:D, hm, qs], in_=st[0:D, :])
                    else:
                        nc.gpsimd.dma_start(out=OT2[D:2 * D, hm, qs], in_=st[0:D, :])
                    nc.sync.dma_start(
                        out=den_dram[qc, 2 * hm + j:2 * hm + j + 1, :],
                        in_=st[D:D + 1, :])

            def norm_pair(qc, hm):
                """1/den for head pair hm of q-chunk qc, then scale OT2 rows."""
                qs = slice(qc * QC, (qc + 1) * QC)
                # reshape [2, 1024] -> [32, 64] so reciprocal uses 32 lanes
                dsb = denpool.tile([32, QC // 16], MD, name="dsb", tag="dsb")
                rsm = denpool.tile([32, QC // 16], F32, name="rsm", tag="rsm")
                hs = slice(2 * hm, 2 * hm + 2)
                den_r = den_dram[qc, hs].rearrange("h (a b) -> (h a) b", a=16)
                rden_r = rden_dram[qc, hs].rearrange("h (a b) -> (h a) b", a=16)
                nc.sync.dma_start(out=dsb, in_=den_r)
                nc.vector.reciprocal(out=rsm, in_=dsb)
                rsb = denpool.tile([32, QC // 16], MD, name="rsb", tag="rsb")
                nc.vector.tensor_copy(out=rsb, in_=rsm)
                nc.sync.dma_start(out=rden_r, in_=rsb)
                rb = rbcpool.tile([128, QC], MD, name="rb")
                for j in range(2):
                    h = 2 * hm + j
                    nc.sync.dma_start(
                        out=rb[j * D:(j + 1) * D, :],
                        in_=rden_dram[qc, h:h + 1, :].to_broadcast((D, QC)))
                nc.vector.tensor_mul(OT2[:, hm, qs], OT2[:, hm, qs], rb)

            def fc_tile(tt, tail=False):
                """Partial fc_out for token tile tt: y[tt] = sum_hm O^T_hm.T@Wo.
                Tail tiles pipeline 4-deep through both psum pools and split
                the psum->bf16 copy across DVE and the now-idle ACT."""
                if tail and tt % 2:
                    ps = popool.tile([128, QC], F32, name="fco", tag="po")
                else:
                    ps = pspool.tile([128, QC], F32, name="fc", tag="ps")
                for nch in range(EMBED // TCH):
                    for hm in range(2):
                        nc.tensor.matmul(
                            ps[:, nch * TCH:(nch + 1) * TCH],
                            OT2[:, hm, tt * 128:(tt + 1) * 128],
                            wo_sb[:, hm, nch * TCH:(nch + 1) * TCH],
                            start=(hm == 0),
                            stop=(hm == 1),
                        )
                ys = ypool.tile([128, EMBED], MD, name="ys")
                if tail:
                    nc.vector.tensor_copy(out=ys[:, 0:TCH], in_=ps[:, 0:TCH])
                    nc.scalar.copy(out=ys[:, TCH:], in_=ps[:, TCH:])
                else:
                    nc.vector.tensor_copy(out=ys, in_=ps)
                if tt % 2 == 0:
                    nc.gpsimd.dma_start(out=y[tt * 128:(tt + 1) * 128, :], in_=ys)
                else:
                    nc.sync.dma_start(out=y[tt * 128:(tt + 1) * 128, :], in_=ys)

            # ---- flat 64-slot schedule over (hm,qc) blocks x 16 key chunks --
            # Scores+exp stream at the ACT pace (one slot ~= 2.3us); a global
            # AV queue trails by LAG slots so the deep es pool absorbs bursts
            # of wedged projection / fc_out work.  Each block's O^T psum pair
            # frees exactly one slot before the next block's first AV needs it.
            BLOCKS = [(0, 0), (1, 0), (0, 1), (1, 1)]
            LAG = 12  # AV(b, m) drains at slot 16b+m+LAG
            # mt halves of K/Q are per-head-pair: block0/2 (hm=0) only need
            # mt0, so mt1 halves defer until just before block1/3 use them.
            # V rows stagger ahead of the AV drain deadlines (slot m+12).
            def qk(wsb, dst, tcb, mt, half):
                return lambda: proj_qk_half(wsb, dst, tcb, mt, half)

            pre = {
                0: [qk(wq_sb, QTs, 0, 1, 0)],
                1: [qk(wq_sb, QTs, 0, 1, 1)],
                2: [qk(wq_sb, QTs, 1, 1, 0)],
                3: [qk(wq_sb, QTs, 1, 1, 1), qk(wk_sb, KTs, 1, 0, 0)],
                4: [qk(wk_sb, KTs, 1, 0, 1)],
                5: [qk(wk_sb, KTs, 0, 1, 0)],
                6: [qk(wk_sb, KTs, 0, 1, 1)],
                7: [qk(wk_sb, KTs, 2, 0, 0)],
                8: [qk(wk_sb, KTs, 2, 0, 1)],
                9: [lambda: proj_v(0, 0)],
                10: [lambda: proj_v(0, 1)],
                11: [lambda: proj_v(0, 2), qk(wk_sb, KTs, 3, 0, 0)],
                12: [qk(wk_sb, KTs, 3, 0, 1), lambda: proj_v(0, 3)],
                13: [lambda: proj_v(1, 0)],
                14: [lambda: proj_v(1, 1)],
                15: [lambda: proj_v(1, 2)],
                16: [lambda: proj_v(1, 3)],
                17: [lambda: proj_v(2, 0)],
                18: [lambda: proj_v(2, 1)],
                19: [qk(wk_sb, KTs, 1, 1, 0)],
                20: [qk(wk_sb, KTs, 1, 1, 1), lambda: proj_v(2, 2)],
                21: [lambda: proj_v(2, 3)],
                22: [lambda: proj_v(3, 0)],
                23: [qk(wk_sb, KTs, 2, 1, 0)],
                24: [qk(wk_sb, KTs, 2, 1, 1), lambda: proj_v(3, 1)],
                25: [lambda: proj_v(3, 2)],
                26: [lambda: proj_v(3, 3)],
                27: [qk(wk_sb, KTs, 3, 1, 0)],
                28: [qk(wk_sb, KTs, 3, 1, 1)],
                29: [qk(wq_sb, QTs, 2, 0, 0)],
                30: [qk(wq_sb, QTs, 2, 0, 1)],
                31: [qk(wq_sb, QTs, 3, 0, 0)],
                32: [qk(wq_sb, QTs, 3, 0, 1)],
                33: [qk(wq_sb, QTs, 2, 1, 0)],
                34: [qk(wq_sb, QTs, 2, 1, 1)],
                37: [qk(wq_sb, QTs, 3, 1, 0)],
                38: [qk(wq_sb, QTs, 3, 1, 1)],
                46: [lambda: fc_tile(0)],
                49: [lambda: fc_tile(2)],
                52: [lambda: fc_tile(4)],
            }
            pos = {}
            pending = []  # (global slot, block, m, es pair)

            def drain(now):
                extra = 2 if now >= 55 else 0
                while pending:
                    s0, b, m, es = pending[0]
                    if s0 + LAG > now:
                        if extra <= 0:
                            break
                        extra -= 1
                    pending.pop(0)
                    hm, qc = BLOCKS[b]
                    if b not in pos:
                        pos[b] = [popool.tile([D + 1, QC], F32, name="po",
                                              tag="po") for _ in range(2)]
                    av(pos[b], hm, m, es)
                    if m == NM - 1:
                        finish_pair(pos.pop(b), hm, qc)
                        norm_pair(qc, hm)

            for s in range(4 * NM):
                b, m = divmod(s, NM)
                for thunk in pre.get(s, ()):
                    thunk()
                hm, qc = BLOCKS[b]
                pending.append((s, b, m, scores(hm, qc, m)))
                drain(s)
            drain(10 ** 9)
            for tt in range(SEQ // 128):
                if tt not in (0, 2, 4):
                    fc_tile(tt, tail=True)

    nc.compile()
    return nc


def shard_inputs(x, Wv, Wk, Wq, Wo):
    """Build the 8 per-core input maps."""
    import ml_dtypes
    wire = ml_dtypes.bfloat16
    in_maps = []
    for c in range(NCORES):
        n, g = divmod(c, GROUPS)
        cols = slice(g * DG, (g + 1) * DG)
        in_maps.append({
            "xT": np.ascontiguousarray(np.asarray(x[n], np.float32).T).astype(wire),
            "wq": np.ascontiguousarray(np.asarray(Wq, np.float32)[:, cols]).astype(wire),
            "wk": np.ascontiguousarray(np.asarray(Wk, np.float32)[:, cols]).astype(wire),
            "wv": np.ascontiguousarray(np.asarray(Wv, np.float32)[:, cols]).astype(wire),
            "wo": np.ascontiguousarray(np.asarray(Wo, np.float32)[cols, :]).astype(wire),
        })
    return in_maps


def kernel(x, Wv, Wk, Wq, Wo, bo):
    global LAST_RESULTS
    x = np.asarray(x, np.float32)
    in_maps = shard_inputs(x, Wv, Wk, Wq, Wo)

    if "nc" not in _CACHED_NC:
        _CACHED_NC["nc"] = build_nc()
    nc = _CACHED_NC["nc"]

    trace = os.environ.get("MHA_TRACE", "0") == "1"
    res = bass_utils.run_bass_kernel_spmd(
        nc, in_maps, core_ids=list(range(NCORES)), trace=trace)
    LAST_RESULTS = res

    bo = np.asarray(bo, np.float32)
    out = np.empty((NB, SEQ, EMBED), np.float32)
    for n in range(NB):
        acc = res.results[n * GROUPS]["y"].astype(np.float32)
        for g in range(1, GROUPS):
            acc = acc + res.results[n * GROUPS + g]["y"].astype(np.float32)
        out[n] = acc + bo[None, :]
    return out


# revision 22
# speedup vs baseline: 1.0240x; 1.0240x over previous
"""Multi-head self-attention Trainium2 kernel (8-core SPMD, full IO).

Problem: x:(2,2048,1024) f32; Wq/Wk/Wv/Wo:(1024,1024); bo:(1024,)
  out = softmax((xWq)(xWk)^T / 8) (xWv) reshaped @ Wo + bo

Sharding: data parallel on batch N=2 x tensor parallel on 16 heads in
4 groups of 4 heads.  Core c handles batch c//4, heads [4*(c%4), 4*(c%4)+4).
Each core computes a partial fc_out product (2048,1024) in bf16; the host
sums the 4 head-group partials per batch in f32 and adds the bias.

v2 schedule notes (why the emission order looks scrambled):
  The scalar engine (ACT) is the kernel's pacer: 128 exp instructions of
  [128,1024] ~= 142us that cannot run anywhere else.  Everything is arranged
  to keep ACT saturated from ~15us onward:
    - attention runs qc-outer (q-chunk 0 fully finishes early) so its
      softmax-normalize + fc_out partial product hide under q-chunk 1;
    - remaining K/V projections are interleaved between the first head-pair's
      score matmuls (PE has slack, ACT does not);
    - fc_out for qc0 is wedged into the scores psum ring mid-way through
      q-chunk 1 (scores run ahead of AV; the 16-deep es pool absorbs the lag);
    - scores are computed TRANSPOSED: S^T[k,q] so exp runs on ACT directly
      from PSUM and the softmax denominator falls out of the ones-column of V
      during the O^T accumulation (row 64 of the [65,1024] psum).
  No max subtraction: scores are ~N(0,1), bounded well inside fp32 exp range.
"""

import os

import numpy as np

import concourse.bass as bass
import concourse.tile as tile
from concourse import bacc, mybir
from concourse import bass_utils

F32 = mybir.dt.float32
BF16 = mybir.dt.bfloat16

EMBED = 1024
SEQ = 2048
NB = 2  # batch
HEADS = 16
D = 64  # head dim
NCORES = 8
GROUPS = 4  # head groups (tensor parallel)
HG = HEADS // GROUPS  # heads per core = 4
DG = HG * D  # dims per core = 256

_MM_DTYPE_NAME = "bfloat16"  # read by test.py
MD = BF16

# set by kernel(); test.py reads exec_time_ns from here
LAST_RESULTS = None
_CACHED_NC = {}

KC = EMBED // 128  # 8 contraction chunks for projections
TCH = 512  # projection token chunk
QC = 1024  # attention q-chunk (one exp instruction per head per m)
NM = SEQ // 128  # 16 key chunks in the attention m-loop
AV_LAG = 3  # AV trails scores by this many m-iterations


def build_nc():
    nc = bacc.Bacc("TRN2", target_bir_lowering=False, debug=False,
                   num_devices=NCORES)

    xT = nc.dram_tensor("xT", (EMBED, SEQ), BF16, kind="ExternalInput").ap()
    wq = nc.dram_tensor("wq", (EMBED, DG), BF16, kind="ExternalInput").ap()
    wk = nc.dram_tensor("wk", (EMBED, DG), BF16, kind="ExternalInput").ap()
    wv = nc.dram_tensor("wv", (EMBED, DG), BF16, kind="ExternalInput").ap()
    wo = nc.dram_tensor("wo", (DG, EMBED), BF16, kind="ExternalInput").ap()
    y = nc.dram_tensor("y", (SEQ, EMBED), BF16, kind="ExternalOutput").ap()
    # DRAM bounce buffers for the softmax denominators: SBUF sources can't be
    # partition-broadcast by DMA, DRAM sources can.
    den_dram = nc.dram_tensor("den_scratch", (2, HG, SEQ // 2), BF16).ap()
    rden_dram = nc.dram_tensor("rden_scratch", (2, HG, SEQ // 2), BF16).ap()

    with tile.TileContext(nc) as tc:
        with (
            tc.tile_pool(name="weights", bufs=1) as wpool,
            tc.tile_pool(name="qk", bufs=1) as qkpool,
            tc.tile_pool(name="vpool", bufs=1) as vpool,
            tc.tile_pool(name="otpool", bufs=1) as otpool,
            tc.tile_pool(name="xchunk", bufs=1) as xpool,
            tc.tile_pool(name="epool", bufs=34) as epool,
            tc.tile_pool(name="stage", bufs=2) as stpool,
            tc.tile_pool(name="den", bufs=2) as denpool,
            tc.tile_pool(name="rbc", bufs=2) as rbcpool,
            tc.tile_pool(name="ystage", bufs=4) as ypool,
            tc.tile_pool(name="psum", bufs=2, space="PSUM") as pspool,
            tc.tile_pool(name="psum_o", bufs=2, space="PSUM") as popool,
        ):
            # ---- weights (wk first: the first score matmul needs K only) ----
            wk_sb = wpool.tile([128, KC, DG], MD)
            wq_sb = wpool.tile([128, KC, DG], MD)
            wv_sb = wpool.tile([128, KC, DG], MD)
            wo_sb = wpool.tile([128, DG // 128, EMBED], MD)
            # (p c) split of the contraction index: per-partition DMA reads are
            # contiguous DRAM rows; the same permutation on both matmul
            # operands leaves the contraction sum unchanged.
            nc.gpsimd.dma_start(out=wk_sb, in_=wk.rearrange("(p c) n -> p c n", p=128))

            QTs = [qkpool.tile([128, 2, TCH], MD, name=f"qt{t}", tag=f"qt{t}")
                   for t in range(4)]
            KTs = [qkpool.tile([128, 2, TCH], MD, name=f"kt{t}", tag=f"kt{t}")
                   for t in range(4)]
            Vs = [vpool.tile([128, 4, HG, D + 1], MD, name=f"v{t}", tag=f"v{t}")
                  for t in range(4)]
            for t in range(4):
                nc.vector.memset(Vs[t][:, :, :, D:D + 1], 1.0)

            xTr = xT.rearrange("(p c) s -> p c s", p=128)
            xbigs = {}

            def load_x(tcb, split=False):
                xc = xpool.tile([128, KC, TCH], MD, name=f"xc{tcb}")
                src_ = xTr[:, :, tcb * TCH:(tcb + 1) * TCH]
                if split:
                    nc.sync.dma_start(out=xc[:, 0:KC // 2], in_=src_[:, 0:KC // 2])
                    nc.gpsimd.dma_start(out=xc[:, KC // 2:], in_=src_[:, KC // 2:])
                else:
                    nc.gpsimd.dma_start(out=xc, in_=src_)
                xbigs[tcb] = xc

            def xa(tcb):
                return xbigs[tcb]

            open_ps = {}

            def proj_qk_half(wsb, dst, tcb, mt, half):
                """Half of a Q^T/K^T projection (4 of 8 contraction chunks).
                Splitting keeps each wedged PE burst under one slot's slack."""
                key = (id(dst), tcb, mt)
                if half == 0:
                    open_ps[key] = pspool.tile([128, QC], F32, name="ps",
                                               tag="ps")
                ps = open_ps[key]
                for kc in range(half * 4, half * 4 + 4):
                    nc.tensor.matmul(
                        ps[:, 0:TCH],
                        wsb[:, kc, mt * 128:(mt + 1) * 128],
                        xa(tcb)[:, kc, :],
                        start=(kc == 0),
                        stop=(kc == KC - 1),
                    )
                if half == 1:
                    nc.vector.tensor_copy(out=dst[tcb][:, mt, :],
                                          in_=ps[:, 0:TCH])
                    del open_ps[key]

            def proj_qk(wsb, dst, tcb, mt):
                proj_qk_half(wsb, dst, tcb, mt, 0)
                proj_qk_half(wsb, dst, tcb, mt, 1)

            def proj_v(tcb, ti):
                """Project one 128-token row of V: [tokens 128, hg, d]."""
                ps = pspool.tile([128, QC], F32, name="psv", tag="ps")
                for kc in range(KC):
                    nc.tensor.matmul(
                        ps[:, 0:DG],
                        xa(tcb)[:, kc, ti * 128:(ti + 1) * 128],
                        wv_sb[:, kc, :],
                        start=(kc == 0),
                        stop=(kc == KC - 1),
                    )
                nc.vector.tensor_copy(
                    out=Vs[tcb][:, ti, :, 0:D],
                    in_=ps[:, 0:DG].rearrange("p (h d) -> p h d", h=HG))

            # ---- head: PE warmup + x0 + K0/Q0/Q1 (hm=0 halves only) so the
            # first exp fires ASAP; everything else streams in behind.
            wrm = wpool.tile([64, 64], MD)
            nc.vector.memset(wrm, 0.0)
            wps = pspool.tile([128, QC], F32, name="wps", tag="ps")
            for i in range(36):
                nc.tensor.matmul(wps[0:64, 0:64], wrm, wrm, start=True, stop=True)
            load_x(0, split=True)
            nc.sync.dma_start(out=wq_sb, in_=wq.rearrange("(p c) n -> p c n", p=128))
            load_x(1, split=True)
            load_x(2)
            load_x(3)
            nc.sync.dma_start(out=wv_sb, in_=wv.rearrange("(p c) n -> p c n", p=128))
            nc.sync.dma_start(out=wo_sb, in_=wo.rearrange("(c p) n -> p c n", p=128))
            proj_qk(wk_sb, KTs, 0, 0)
            proj_qk(wq_sb, QTs, 0, 0)
            proj_qk(wq_sb, QTs, 1, 0)

            # OT2[p, hm, q]: partition p = 64*j + d for head h = 2*hm + j.
            # This matches wo_sb's row layout so fc_out contracts K=128/pair.
            OT2 = otpool.tile([128, 2, SEQ], MD)

            def scores(hm, qc, m):
                """S^T for key chunk m, both heads of the pair, + exp -> es.
                ha-major emission: the j0/j1 matmuls land in different PE
                row-groups (partitions 0-63 vs 64-127), giving the engine a
                chance to overlap them."""
                pss = [pspool.tile([128, QC], F32, name="ps", tag="ps")
                       for _ in range(2)]
                for ha in range(QC // TCH):
                    for j in range(2):
                        nc.tensor.matmul(
                            pss[j][:, ha * TCH:(ha + 1) * TCH],
                            KTs[m // 4][j * D:(j + 1) * D, hm,
                                        (m % 4) * 128:(m % 4 + 1) * 128],
                            QTs[(QC // TCH) * qc + ha][j * D:(j + 1) * D, hm, :],
                            start=True,
                            stop=True,
                        )
                es = []
                for j in range(2):
                    e = epool.tile([128, QC], MD, name="es")
                    nc.scalar.activation(
                        out=e, in_=pss[j],
                        func=mybir.ActivationFunctionType.Exp,
                        scale=1.0 / np.sqrt(D),
                    )
                    es.append(e)
                return es

            def av(po, hm, m, es):
                for j in range(2):
                    for ha in range(QC // TCH):
                        nc.tensor.matmul(
                            po[j][:, ha * TCH:(ha + 1) * TCH],
                            Vs[m // 4][:, m % 4, 2 * hm + j, :],
                            es[j][:, ha * TCH:(ha + 1) * TCH],
                            start=(m == 0),
                            stop=(m == NM - 1),
                        )

            def finish_pair(po, hm, qc):
                """po -> OT2 rows (bf16) + denominator row -> DRAM."""
                qs = slice(qc * QC, (qc + 1) * QC)
                for j in range(2):
                    st = stpool.tile([D + 1, QC], MD, name="st", tag="st")
                    nc.vector.tensor_copy(out=st, in_=po[j][0:D + 1, :])
                    if j == 0:
                        # partition-aligned: row 64 (den) DMAs out, dims DMA'd
                        # with the j=1 hop below would also work but DVE->OT2
                        # direct is cheaper for j=0.
                        nc.gpsimd.dma_start(out=OT2[0:D, hm, qs], in_=st[0:D, :])
                    else:
                        nc.gpsimd.dma_start(out=OT2[D:2 * D, hm, qs], in_=st[0:D, :])
                    nc.sync.dma_start(
                        out=den_dram[qc, 2 * hm + j:2 * hm + j + 1, :],
                        in_=st[D:D + 1, :])

            def norm_pair(qc, hm):
                """1/den for head pair hm of q-chunk qc, then scale OT2 rows."""
                qs = slice(qc * QC, (qc + 1) * QC)
                # reshape [2, 1024] -> [32, 64] so reciprocal uses 32 lanes
                dsb = denpool.tile([32, QC // 16], MD, name="dsb", tag="dsb")
                rsm = denpool.tile([32, QC // 16], F32, name="rsm", tag="rsm")
                hs = slice(2 * hm, 2 * hm + 2)
                den_r = den_dram[qc, hs].rearrange("h (a b) -> (h a) b", a=16)
                rden_r = rden_dram[qc, hs].rearrange("h (a b) -> (h a) b", a=16)
                nc.sync.dma_start(out=dsb, in_=den_r)
                nc.vector.reciprocal(out=rsm, in_=dsb)
                rsb = denpool.tile([32, QC // 16], MD, name="rsb", tag="rsb")
                nc.vector.tensor_copy(out=rsb, in_=rsm)
                nc.sync.dma_start(out=rden_r, in_=rsb)
                rb = rbcpool.tile([128, QC], MD, name="rb")
                for j in range(2):
                    h = 2 * hm + j
                    nc.sync.dma_start(
                        out=rb[j * D:(j + 1) * D, :],
                        in_=rden_dram[qc, h:h + 1, :].to_broadcast((D, QC)))
                nc.vector.tensor_mul(OT2[:, hm, qs], OT2[:, hm, qs], rb)

            def fc_tile(tt, tail=False):
                """Partial fc_out for token tile tt: y[tt] = sum_hm O^T_hm.T@Wo.
                Tail tiles pipeline 4-deep through both psum pools and split
                the psum->bf16 copy across DVE and the now-idle ACT."""
                if tail and tt % 2:
                    ps = popool.tile([128, QC], F32, name="fco", tag="po")
                else:
                    ps = pspool.tile([128, QC], F32, name="fc", tag="ps")
                for nch in range(EMBED // TCH):
                    for hm in range(2):
                        nc.tensor.matmul(
                            ps[:, nch * TCH:(nch + 1) * TCH],
                            OT2[:, hm, tt * 128:(tt + 1) * 128],
                            wo_sb[:, hm, nch * TCH:(nch + 1) * TCH],
                            start=(hm == 0),
                            stop=(hm == 1),
                        )
                ys = ypool.tile([128, EMBED], MD, name="ys")
                if tail:
                    nc.vector.tensor_copy(out=ys[:, 0:TCH], in_=ps[:, 0:TCH])
                    nc.scalar.copy(out=ys[:, TCH:], in_=ps[:, TCH:])
                else:
                    nc.vector.tensor_copy(out=ys, in_=ps)
                if tt % 2 == 0:
                    nc.gpsimd.dma_start(out=y[tt * 128:(tt + 1) * 128, :], in_=ys)
                else:
                    nc.sync.dma_start(out=y[tt * 128:(tt + 1) * 128, :], in_=ys)

            # ---- flat 64-slot schedule over (hm,qc) blocks x 16 key chunks --
            # Scores+exp stream at the ACT pace (one slot ~= 2.3us); a global
            # AV queue trails by LAG slots so the deep es pool absorbs bursts
            # of wedged projection / fc_out work.  Each block's O^T psum pair
            # frees exactly one slot before the next block's first AV needs it.
            BLOCKS = [(0, 0), (1, 0), (0, 1), (1, 1)]
            LAG = 12  # AV(b, m) drains at slot 16b+m+LAG
            # mt halves of K/Q are per-head-pair: block0/2 (hm=0) only need
            # mt0, so mt1 halves defer until just before block1/3 use them.
            # V rows stagger ahead of the AV drain deadlines (slot m+12).
            pre = {
                4: [lambda: proj_qk(wk_sb, KTs, 1, 0)],
                5: [lambda: proj_v(0, 0)],
                6: [lambda: proj_v(0, 1)],
                7: [lambda: proj_v(0, 2)],
                8: [lambda: proj_qk(wk_sb, KTs, 2, 0)],
                9: [lambda: proj_v(0, 3)],
                10: [lambda: proj_qk(wq_sb, QTs, 0, 1)],
                11: [lambda: proj_qk(wq_sb, QTs, 1, 1)],
                12: [lambda: proj_qk(wk_sb, KTs, 3, 0)],
                13: [lambda: proj_v(1, 0), lambda: proj_qk(wk_sb, KTs, 0, 1)],
                14: [lambda: proj_v(1, 1)],
                15: [lambda: proj_v(1, 2)],
                16: [lambda: proj_v(1, 3)],
                17: [lambda: proj_v(2, 0)],
                18: [lambda: proj_v(2, 1), lambda: proj_qk(wk_sb, KTs, 1, 1)],
                19: [lambda: proj_v(2, 2)],
                20: [lambda: proj_v(2, 3)],
                21: [lambda: proj_v(3, 0)],
                22: [lambda: proj_v(3, 1), lambda: proj_qk(wk_sb, KTs, 2, 1)],
                23: [lambda: proj_v(3, 2)],
                24: [lambda: proj_v(3, 3)],
                27: [lambda: proj_qk(wk_sb, KTs, 3, 1)],
                30: [lambda: proj_qk(wq_sb, QTs, 2, 0)],
                31: [lambda: proj_qk(wq_sb, QTs, 3, 0)],
                36: [lambda: proj_qk(wq_sb, QTs, 2, 1)],
                40: [lambda: proj_qk(wq_sb, QTs, 3, 1)],
                46: [lambda: fc_tile(0)],
                49: [lambda: fc_tile(2)],
                52: [lambda: fc_tile(4)],
            }
            pos = {}
            pending = []  # (global slot, block, m, es pair)

            def drain(now):
                extra = 2 if now >= 50 else 0
                while pending:
                    s0, b, m, es = pending[0]
                    if s0 + LAG > now:
                        if extra <= 0:
                            break
                        extra -= 1
                    pending.pop(0)
                    hm, qc = BLOCKS[b]
                    if b not in pos:
                        pos[b] = [popool.tile([D + 1, QC], F32, name="po",
                                              tag="po") for _ in range(2)]
                    av(pos[b], hm, m, es)
                    if m == NM - 1:
                        finish_pair(pos.pop(b), hm, qc)
                        norm_pair(qc, hm)

            for s in range(4 * NM):
                b, m = divmod(s, NM)
                for thunk in pre.get(s, ()):
                    thunk()
                hm, qc = BLOCKS[b]
                pending.append((s, b, m, scores(hm, qc, m)))
                drain(s)
            drain(10 ** 9)
            for tt in range(SEQ // 128):
                if tt not in (0, 2, 4):
                    fc_tile(tt, tail=True)

    nc.compile()
    return nc


def shard_inputs(x, Wv, Wk, Wq, Wo):
    """Build the 8 per-core input maps."""
    import ml_dtypes
    wire = ml_dtypes.bfloat16
    in_maps = []
    for c in range(NCORES):
        n, g = divmod(c, GROUPS)
        cols = slice(g * DG, (g + 1) * DG)
        in_maps.append({
            "xT": np.ascontiguousarray(np.asarray(x[n], np.float32).T).astype(wire),
            "wq": np.ascontiguousarray(np.asarray(Wq, np.float32)[:, cols]).astype(wire),
            "wk": np.ascontiguousarray(np.asarray(Wk, np.float32)[:, cols]).astype(wire),
            "wv": np.ascontiguousarray(np.asarray(Wv, np.float32)[:, cols]).astype(wire),
            "wo": np.ascontiguousarray(np.asarray(Wo, np.float32)[cols, :]).astype(wire),
        })
    return in_maps


def kernel(x, Wv, Wk, Wq, Wo, bo):
    global LAST_RESULTS
    x = np.asarray(x, np.float32)
    in_maps = shard_inputs(x, Wv, Wk, Wq, Wo)

    if "nc" not in _CACHED_NC:
        _CACHED_NC["nc"] = build_nc()
    nc = _CACHED_NC["nc"]

    trace = os.environ.get("MHA_TRACE", "0") == "1"
    res = bass_utils.run_bass_kernel_spmd(
        nc, in_maps, core_ids=list(range(NCORES)), trace=trace)
    LAST_RESULTS = res

    bo = np.asarray(bo, np.float32)
    out = np.empty((NB, SEQ, EMBED), np.float32)
    for n in range(NB):
        acc = res.results[n * GROUPS]["y"].astype(np.float32)
        for g in range(1, GROUPS):
            acc = acc + res.results[n * GROUPS + g]["y"].astype(np.float32)
        out[n] = acc + bo[None, :]
    return out
